# revision 37
# baseline (speedup 1.0000x reference)
"""Trainium2 Bass kernel for DecoupledAttentionAggregation GNN message passing.

Strategy (per sharding hint): destination nodes are dealt round-robin (after a
global degree-profile sort) across 8 cores; each core owns all edges into its
nodes, does local segment softmax / top-k / 3-group aggregation, and writes its
own output rows. The host shards/permutes/packs inputs; the device does all the
per-edge math (matmuls, softmax, top-k, weighted aggregation).

Device layout (v2, bf16): each core's nodes are arranged into 128-row blocks. A
node-row r lives on SBUF partition r%128; its (per-group padded) edge slots
occupy consecutive f-columns of its block; blocks in a chunk share one width W,
so per-destination softmax/top-k are free-dim windowed ops.

Per-edge compute: ONE matmul per 128-slot tile with a fused weight matrix:
  A = [h_hi(64) | ea_hi(32) | pc(4) | ones(1)]  (bf16, 101 rows)
  W = [whW|a_r ; weW|a_e ; 0|I4 ; bias|0]       ([101, 68])
giving messages (psum cols 0:64) and complete per-head attention scores
(cols 64:68; pc = dest-node score component, gathered host-side, with -1e30
folded in for pad slots). Softmax runs unnormalized (1/den is applied by the
scalar engine at aggregation-PSUM eviction). Messages are evicted
feature-major so the alpha*msg multiply runs in DVE bf16 2x mode. Top-k uses
a fused knockout-select custom DVE op. Output is written bf16 and upcast on
the host.
"""

import sys

sys.path.insert(0, "/opt/trn_rl_repo")

import numpy as np
import ml_dtypes

import concourse.bacc as bacc
import concourse.bass as bass
import concourse.mybir as mybir
import concourse.tile as tile
from concourse import bass_utils
from concourse import dve_ops as _dvo
from concourse.dve_spec import (
    Spec, Src0, Src1, Zero, C0, maxx, select, lower as _dve_lower,
    _has_src1,
)
from concourse.dve_uop import DveOpSpec

BF16 = mybir.dt.bfloat16
F32 = mybir.dt.float32

NCORES = 8
TOPK = 10
NEG = -1.0e30
H = 64
ED = 32
NH = 4
NROWS_A = 96      # h(64) + ea(32); %16==0 so the A DMA spreads over 16 engines
CHUNK_COLS = 128  # max f-columns per chunk (x128 slots)
PB = 7            # msg/score matmul tiles per PSUM buffer (7*68*4B < 2KB)
KAGG = 7          # aggregation matmul batch (moving cols = KAGG*64 <= 512)
GP_MULT_EVERY = 10 ** 9  # every Nth chunk's alpha*msg multiply runs on GpSimd


def _register_dve_op(name, spec, subdim=False):
    existing = {op.name: op for op in _dvo.OPS}
    if name in existing:
        return existing[name]
    row = _dvo._CUSTOM_DVE_ROW_BASE + len(_dvo.OPS)
    assert row < 0x20
    shas = {}
    for ver in ("v3", "v4"):
        s = DveOpSpec(name=name, opcode=row, uops=_dve_lower(spec, ver=ver),
                      rd1_en=_has_src1(spec))
        shas[ver] = s.sha(ver)
    op = _dvo.DveOp(name, spec, subdim=subdim, uops_sha=shas)
    _dvo.OPS.append(op)
    _dvo.CUSTOM_DVE_SPECS[name] = spec
    _dvo._SUB_OPCODE_FOR_NAME[name] = row
    return op


# score eviction: out = leaky_relu(x + pc) = max(v, s0*v), v = x + pc
LEAKY_EVICT = _register_dve_op(
    "ANT_GNN_LEAKY2",
    Spec(body=maxx(Src0 + Src1, (Src0 + Src1) * C0),
         reference=lambda in0, in1, s0: np.maximum(in0 + in1, (in0 + in1) * s0)
         .astype(np.float32)),
)
# top-k knockout: out = x if x < m else 0
KNOCK = _register_dve_op(
    "ANT_GNN_KNOCK",
    Spec(body=select(Src0 < Src1, Src0, Zero),
         reference=lambda in0, in1: np.where(in0 < in1, in0, 0.0).astype(np.float32)),
)
# top-k threshold keep: out = x if x >= m else 0
THRESH = _register_dve_op(
    "ANT_GNN_THRESH",
    Spec(body=select(Src0 >= Src1, Src0, Zero),
         reference=lambda in0, in1: np.where(in0 >= in1, in0, 0.0).astype(np.float32)),
)


def _plan_and_pack(h, edge_index, edge_attr, node_labels, attn_w, whW, whb, weW, web):
    """Host-side sharding/packing. Returns (plan, in_maps, assemble_info)."""
    N = h.shape[0]
    row = np.asarray(edge_index[0], dtype=np.int64)
    col = np.asarray(edge_index[1], dtype=np.int64)
    labels = np.asarray(node_labels)

    # edge groups: 0=same, 1=diff, 2=unlabeled
    lr, lc = labels[row], labels[col]
    g = np.where(
        (lr == lc) & (lr != -1),
        0,
        np.where((lr != lc) & (lr != -1) & (lc != -1), 1, 2),
    ).astype(np.int64)

    deg_g = np.zeros((N, 3), np.int64)
    np.add.at(deg_g, (col, g), 1)

    # Global sort nodes by per-group degree profile, deal round-robin to cores.
    perm_global = np.lexsort((-deg_g[:, 2], -deg_g[:, 1], -deg_g[:, 0]))
    D = (N + NCORES - 1) // NCORES
    NB = (D + 127) // 128
    R = NB * 128

    node_of_row = np.full((NCORES, R), -1, np.int64)
    for c in range(NCORES):
        nodes_c = perm_global[c::NCORES]
        node_of_row[c, : len(nodes_c)] = nodes_c

    # canonical per-block per-group widths (max over cores)
    dg_rows = np.zeros((NCORES, R, 3), np.int64)
    for c in range(NCORES):
        valid = node_of_row[c] >= 0
        dg_rows[c, valid] = deg_g[node_of_row[c, valid]]
    Wg = dg_rows.reshape(NCORES, NB, 128, 3).max(axis=(0, 2))  # [NB,3]
    Wtot = Wg.sum(1)

    # Reorder blocks by Wtot desc so chunks have uniform width.
    border = np.argsort(-Wtot, kind="stable")
    Wg = Wg[border]
    Wtot = Wtot[border]
    rowperm = (border[:, None] * 128 + np.arange(128)[None, :]).reshape(-1)
    node_of_row = node_of_row[:, rowperm]

    # chunks: greedy fill; every block padded (in group 2) to the chunk width
    chunks = []
    b0 = 0
    while b0 < NB:
        Wc = int(Wtot[b0])
        if Wc == 0:
            break
        nmax = max(1, CHUNK_COLS // max(Wc, 1))
        b1 = min(b0 + nmax, NB)
        while b1 > b0 + 1 and Wtot[b1 - 1] == 0:
            b1 -= 1
        chunks.append((b0, b1, Wc))
        b0 = b1
    Wg = Wg.copy()
    for (b0, b1, Wc) in chunks:
        Wg[b0:b1, 2] += Wc - Wtot[b0:b1]
    Wtot = Wg.sum(1)
    Fb_off = np.concatenate([[0], np.cumsum(Wtot)])
    F = int(Fb_off[-1])

    in_maps = [dict() for _ in range(NCORES)]

    core_of_node = np.empty(N, np.int64)
    row_of_node = np.empty(N, np.int64)
    for c in range(NCORES):
        valid = node_of_row[c] >= 0
        core_of_node[node_of_row[c, valid]] = c
        row_of_node[node_of_row[c, valid]] = np.nonzero(valid)[0]

    e_core = core_of_node[col]
    e_row = row_of_node[col]

    # weights (mean over heads folded as 0.25)
    aw = np.asarray(attn_w, np.float32) * 0.25
    a_r, a_c, a_e = aw[:H], aw[H : 2 * H], aw[2 * H :]
    whW32 = np.asarray(whW, np.float32)
    weW32 = np.asarray(weW, np.float32)
    bias_comb = np.asarray(whb, np.float32) + np.asarray(web, np.float32)

    bf = ml_dtypes.bfloat16
    has_bias = bool(np.any(bias_comb != 0))
    nrows = NROWS_A if not has_bias else 112
    # fused weight matrix [nrows, 68]
    W_mat = np.zeros((nrows, H + NH), np.float32)
    W_mat[:H, :H] = whW32
    W_mat[H : H + ED, :H] = weW32
    W_mat[:H, H:] = a_r
    W_mat[H : H + ED, H:] = a_e
    if has_bias:
        W_mat[H + ED, :H] = bias_comb
    W_mat = W_mat.astype(bf)
    ident = np.eye(128, dtype=bf)

    h_hi = np.asarray(h, np.float32).astype(bf)
    ea_hi = np.asarray(edge_attr, np.float32).astype(bf)
    pc_nodes = np.asarray(h, np.float32) @ a_c  # [N, 4] f32

    # pad-slot kill vector: v such that v @ a_e = -1e30 per head (pad slots
    # carry v in their ea rows, so their scores leave the matmul at -1e30)
    a_e_b = a_e.astype(bf).astype(np.float32)  # [ED, NH]
    v_kill = np.linalg.lstsq(a_e_b.T, np.full(NH, NEG, np.float64), rcond=None)[0]
    v_kill = v_kill.astype(bf)
    chk = v_kill.astype(np.float32) @ a_e_b
    assert np.all(chk < -1e29), chk
    assert np.all(np.abs(v_kill.astype(np.float32) @ W_mat[H : H + ED, :H]
                         .astype(np.float32)) < 1e36)

    goff = np.zeros((NB, 4), np.int64)
    goff[:, 1] = Wg[:, 0]
    goff[:, 2] = Wg[:, 0] + Wg[:, 1]
    goff[:, 3] = Wtot

    e_p = e_row & 127

    # order edges by (core, row, group); position within run -> slot column
    es = np.lexsort((g, e_row, e_core))
    key = (e_core[es] * R + e_row[es]) * 4 + g[es]
    runs_start = np.r_[True, key[1:] != key[:-1]]
    run_id = np.cumsum(runs_start) - 1
    first_of = np.full(run_id[-1] + 1, len(es), np.int64)
    np.minimum.at(first_of, run_id, np.arange(len(es)))
    pos = np.arange(len(es)) - first_of[run_id]
    e_block = e_row >> 7
    fcol = Fb_off[e_block[es]] + goff[e_block[es], g[es]] + pos
    assert (pos < Wg[e_block[es], g[es]]).all()

    chunk_meta = []
    for (b0, b1, Wc) in chunks:
        chunk_meta.append(
            dict(
                b0=b0,
                b1=b1,
                cols=int(Fb_off[b1] - Fb_off[b0]),
                col_off=int(Fb_off[b0]),
                nrows=(b1 - b0) * 128,
                row_off=b0 * 128,
                W=int(Wc),
            )
        )

    # pre-transposed source tables; h pad col = 0, ea pad col = v_kill
    hT_hi = np.ascontiguousarray(
        np.concatenate([h_hi, np.zeros((1, H), bf)]).T.view(np.uint16))
    eaT_hi = np.ascontiguousarray(
        np.concatenate([ea_hi, v_kill[None, :]]).T.view(np.uint16))
    E = len(row)

    for c in range(NCORES):
        mask = e_core[es] == c
        ef = es[mask]
        fc = fcol[mask]
        pp = e_p[ef]
        eid_grid = np.full((F, 128), -1, np.int64)
        eid_grid[fc, pp] = ef
        real = eid_grid >= 0
        flat_eid = eid_grid.reshape(-1)
        flat_real = real.reshape(-1)
        idxr = np.where(flat_real, flat_eid, E)  # E -> v_kill row
        rsrc = np.where(flat_real, row[np.maximum(flat_eid, 0)], N)  # N -> zero row

        A = np.zeros((nrows, F * 128), bf)
        A16 = A.view(np.uint16)
        A16[:H, :] = hT_hi[:, rsrc]
        A16[H : H + ED, :] = eaT_hi[:, idxr]
        if has_bias:
            A[H + ED, :] = bf(1.0)

        # dest-node score component per (lane, block); -1e30 kills unused rows
        nrw = node_of_row[c]
        pcb = np.full((R, NH), NEG, np.float32)
        valid = nrw >= 0
        pcb[valid] = pc_nodes[nrw[valid]]
        pcb = np.ascontiguousarray(
            pcb.reshape(NB, 128, NH).transpose(1, 0, 2).reshape(128, NB * NH))

        m = in_maps[c]
        m["A"] = A
        m["Wm"] = W_mat
        m["ident"] = ident
        m["pcb"] = pcb

    plan = dict(N=N, D=D, NB=NB, R=R, F=F, Wg=Wg, Wtot=Wtot, Fb_off=Fb_off,
                goff=goff, chunks=chunk_meta, nrows=nrows)
    assemble = dict(node_of_row=node_of_row, R=R)
    return plan, in_maps, assemble


def _build_program(plan):
    NB, F, R = plan["NB"], plan["F"], plan["R"]
    Fb_off = plan["Fb_off"]
    chunks = plan["chunks"]
    nrows = plan["nrows"]

    nc = bacc.Bacc(
        "TRN2",
        target_bir_lowering=False,
        debug=False,
        enable_asserts=False,
        num_devices=NCORES,
    )

    A_d = nc.dram_tensor("A", [nrows, F * 128], BF16, kind="ExternalInput")
    W_d = nc.dram_tensor("Wm", [nrows, H + NH], BF16, kind="ExternalInput")
    id_d = nc.dram_tensor("ident", [128, 128], BF16, kind="ExternalInput")
    pcb_d = nc.dram_tensor("pcb", [128, NB * NH], F32, kind="ExternalInput")
    # output transposed per lane: row for node (bi*128+p) lives at [p, bi, :]
    out_d = nc.dram_tensor("out", [128, NB * 3 * H], BF16, kind="ExternalOutput")
    maxnblk = max(cm["nrows"] // 128 for cm in chunks)

    with tile.TileContext(nc) as tc:
        with (
            tc.tile_pool(name="const", bufs=1) as cpool,
            tc.tile_pool(name="dma", bufs=2) as dpool,
            tc.tile_pool(name="work", bufs=3) as wpool,
            tc.tile_pool(name="psum_m", bufs=6, space="PSUM") as pmpool,
            tc.tile_pool(name="psum_o", bufs=2, space="PSUM") as popool,
        ):
            W_s = cpool.tile([nrows, H + NH], BF16, tag="wm")
            id_s = cpool.tile([128, 128], BF16, tag="ident")
            pcb_s = cpool.tile([128, NB, NH], F32, tag="pcb")
            nc.sync.dma_start(out=W_s[:], in_=W_d.ap())
            nc.sync.dma_start(out=id_s[:], in_=id_d.ap())
            nc.sync.dma_start(out=pcb_s[:], in_=pcb_d.ap())

            def frontend(mi, cm):
                cols = cm["cols"]
                nblk = cm["nrows"] // 128
                Wc = cm["W"]
                c0 = cm["col_off"]
                nslots = cols * 128

                A_sb = dpool.tile([nrows, nslots], BF16, tag="A")
                nc.sync.dma_start(
                    out=A_sb[:], in_=A_d.ap()[:, c0 * 128 : c0 * 128 + nslots]
                )



                # per-tile fused msg+score matmul; evict msg (t-major) + scores
                msg_sb = wpool.tile([128, cols, H], BF16, tag="msg")
                sraw_sb = wpool.tile([128, cols, NH], F32, tag="sraw")
                for t0 in range(0, cols, PB):
                    tb = min(PB, cols - t0)
                    pm = pmpool.tile([128, PB * (H + NH)], F32, tag="psum_msg")
                    for j in range(tb):
                        t = t0 + j
                        sl = slice(t * 128, (t + 1) * 128)
                        nc.tensor.matmul(
                            out=pm[:, j * (H + NH) : (j + 1) * (H + NH)],
                            lhsT=A_sb[:, sl], rhs=W_s[:],
                            start=True, stop=True,
                        )
                    pmv = pm[:].rearrange("p (t f) -> p t f", f=H + NH)
                    nc.scalar.activation(
                        out=msg_sb[:, t0 : t0 + tb, :],
                        in_=pmv[:, :tb, :H],
                        func=mybir.ActivationFunctionType.Relu,
                    )
                    # scores evict: split at block boundaries so in1 (the
                    # per-block dest component) stays a rank-3 broadcast
                    u0 = t0
                    while u0 < t0 + tb:
                        blk = u0 // Wc
                        u1 = min((blk + 1) * Wc, t0 + tb)
                        nc.vector._custom_dve(
                            LEAKY_EVICT,
                            out=sraw_sb[:, u0:u1, :],
                            in0=pmv[:, u0 - t0 : u1 - t0, H:],
                            in1=pcb_s[:, cm["b0"] + blk, :]
                            .unsqueeze(1).to_broadcast([128, u1 - u0, NH]),
                            s0=0.2,
                        )
                        u0 = u1

                # scores: head-sum, exp, den, inv
                s_sb = wpool.tile([128, cols], F32, tag="scores")
                nc.vector.tensor_reduce(out=s_sb[:], in_=sraw_sb[:],
                                        axis=mybir.AxisListType.X,
                                        op=mybir.AluOpType.add)
                ex_sb = wpool.tile([128, cols], F32, tag="ex")
                nc.scalar.activation(out=ex_sb[:], in_=s_sb[:],
                                     func=mybir.ActivationFunctionType.Exp)
                sW = lambda ap: ap.rearrange("p (b w) -> p b w", w=Wc)
                den_sb = wpool.tile([128, nblk], F32, tag="den")
                nc.vector.tensor_reduce(out=den_sb[:], in_=sW(ex_sb[:]),
                                        axis=mybir.AxisListType.X,
                                        op=mybir.AluOpType.add)
                nc.vector.tensor_scalar_add(den_sb[:], den_sb[:], 1e-30)
                inv_sb = wpool.tile([128, nblk], F32, tag="invden")
                nc.vector.reciprocal(out=inv_sb[:], in_=den_sb[:])

                # top-k threshold: iterative max extraction on a copy of ex
                work_sb = wpool.tile([128, cols], F32, tag="work")
                nc.vector.tensor_copy(out=work_sb[:], in_=ex_sb[:])
                m_sb = wpool.tile([128, nblk], F32, tag="mx")
                mbc = m_sb[:].unsqueeze(2).to_broadcast([128, nblk, Wc])
                for it in range(TOPK):
                    nc.vector.tensor_reduce(out=m_sb[:], in_=sW(work_sb[:]),
                                            axis=mybir.AxisListType.X,
                                            op=mybir.AluOpType.max)
                    if it < TOPK - 1:
                        nc.vector._custom_dve(
                            KNOCK, out=sW(work_sb[:]), in0=sW(work_sb[:]), in1=mbc)

                # alpha (bf16, unnormalized): ex if ex >= theta else 0, then
                # duplicated in pairs so the multiply's in1 packs two bf16
                # per 32-bit read -> 2x mode
                al1_sb = wpool.tile([128, cols], BF16, tag="alpha1")
                nc.vector._custom_dve(
                    THRESH, out=sW(al1_sb[:]), in0=sW(ex_sb[:]), in1=mbc)
                al_sb = wpool.tile([128, cols, 2], BF16, tag="alpha")
                nc.vector.tensor_copy(
                    out=al_sb[:],
                    in_=al1_sb[:].unsqueeze(2).to_broadcast([128, cols, 2]),
                )

                # weighted messages: bf16 2x multiply (t-major, paired alpha);
                # a column slice of each chunk runs in parallel on GpSimd
                wmsg_sb = wpool.tile([128, cols, H], BF16, tag="wmsg")
                gcols = 0  # GpSimd TT is too slow/high-latency to help here
                dcols = cols - gcols
                mpair = msg_sb[:, :dcols].rearrange("p t (fp i) -> p t fp i", i=2)
                wpair = wmsg_sb[:, :dcols].rearrange("p t (fp i) -> p t fp i", i=2)
                apair = al_sb[:, :dcols].unsqueeze(2).to_broadcast(
                    [128, dcols, H // 2, 2])
                nc.vector.tensor_tensor(out=wpair, in0=mpair, in1=apair,
                                        op=mybir.AluOpType.mult)
                if gcols:
                    nc.gpsimd.tensor_tensor(
                        out=wmsg_sb[:, dcols:], in0=msg_sb[:, dcols:],
                        in1=al1_sb[:, dcols:].unsqueeze(2)
                        .to_broadcast([128, gcols, H]),
                        op=mybir.AluOpType.mult)
                return wmsg_sb, inv_sb

            def aggregate(cm, wmsg_sb, inv_sb):
                # aggregation per block/group: PSUM-accumulated identity
                # matmuls, batched KAGG columns per instruction via a
                # broadcast (stride-0) PSUM out AP; 1/den applied by the
                # scalar engine at eviction. Emitted one chunk behind the
                # front-end so PE/Scalar never stall on the current chunk's
                # softmax tail.
                nblk = cm["nrows"] // 128
                c0 = cm["col_off"]
                osb = wpool.tile([128, maxnblk, 3 * H], BF16, tag="outsb")
                for b in range(nblk):
                    gb = plan["Wg"][cm["b0"] + b]
                    bc0 = int(Fb_off[cm["b0"] + b] - c0)
                    po = popool.tile([128, 3 * H], F32, tag="psum_out")
                    off = 0
                    for gi in range(3):
                        wgi = int(gb[gi])
                        if wgi == 0:
                            continue
                        j0 = bc0 + off
                        pog = po[:, gi * H : (gi + 1) * H]
                        nc.tensor.matmul(
                            out=pog, lhsT=id_s[:], rhs=wmsg_sb[:, j0, :],
                            start=True, stop=(wgi == 1),
                        )
                        j = 1
                        while j < wgi:
                            k = min(KAGG, wgi - j)
                            nc.tensor.matmul(
                                out=pog.unsqueeze(1).to_broadcast([128, k, H]),
                                lhsT=id_s[:],
                                rhs=wmsg_sb[:, j0 + j : j0 + j + k, :],
                                start=False, stop=(j + k == wgi),
                                skip_group_check=True,
                            )
                            j += k
                        off += wgi
                    # dead-group ranges carry junk x inv here; the host zeroes
                    # them (it knows Wg), so no device memset is needed
                    nc.scalar.activation(
                        out=osb[:, b, :], in_=po[:],
                        func=mybir.ActivationFunctionType.Copy,
                        scale=inv_sb[:, b : b + 1],
                    )
                # issue via GpSimd SWDGE so neither HWDGE ring's in-order
                # queue stalls A-prefetches behind an output DMA
                nc.gpsimd.dma_start(
                    out=out_d.ap()[
                        :, cm["b0"] * 3 * H : (cm["b0"] + nblk) * 3 * H
                    ],
                    in_=osb[:, :nblk, :],
                )

            pending = None
            for mi, cm in enumerate(chunks):
                state = frontend(mi, cm)
                if pending is not None:
                    aggregate(pending[0], *pending[1])
                pending = (cm, state)
            aggregate(pending[0], *pending[1])

    nc.compile()
    return nc


_LAST = {}


def kernel(**inputs):
    import time

    t0 = time.time()
    h = np.asarray(inputs["h"])
    plan, in_maps, assemble = _plan_and_pack(
        h,
        np.asarray(inputs["edge_index"]),
        np.asarray(inputs["edge_attr"]),
        np.asarray(inputs["node_labels"]),
        np.asarray(inputs["attn_w"]),
        np.asarray(inputs["whW"]),
        np.asarray(inputs["whb"]),
        np.asarray(inputs["weW"]),
        np.asarray(inputs["web"]),
    )
    t1 = time.time()
    nc = _build_program(plan)
    t2 = time.time()
    _LAST.update(nc=nc, in_maps=in_maps, plan=plan, assemble=assemble)
    res = bass_utils.run_bass_kernel_spmd(nc, in_maps, core_ids=list(range(NCORES)))
    t3 = time.time()
    print(f"kernel phases: pack {t1-t0:.1f}s build+compile {t2-t1:.1f}s run {t3-t2:.1f}s",
          flush=True)
    N = plan["N"]
    out = np.zeros((N, 3 * H), np.float32)
    nr = assemble["node_of_row"]
    Wg = plan["Wg"]
    dead = Wg == 0  # [NB, 3]; those (block, group) ranges carry device junk
    for c in range(NCORES):
        o = np.asarray(res.results[c]["out"]).astype(np.float32)
        # [128, NB*3H] -> [R, 3H]: node row bi*128+p lives at o[p, bi*3H:...]
        o = o.reshape(128, plan["NB"], 3 * H)
        for gi in range(3):
            o[:, dead[:, gi], gi * H : (gi + 1) * H] = 0.0
        o = o.transpose(1, 0, 2).reshape(-1, 3 * H)
        valid = nr[c] >= 0
        out[nr[c, valid]] = o[valid]
    return out
